# revision 64
# baseline (speedup 1.0000x reference)
"""Causal self-attention (B=2, S=2048, E=1024, H=16) on 8 Trainium2 cores.

Sharding: core c in 0..7 handles batch b = c//4 and the 4 heads
[4*(c%4), 4*(c%4)+4).  The host pre-transposes x[b], pre-slices the QKV
weights column-wise / Wo row-wise per core, and pre-swizzles each
weight into the device SBUF layout so every weight load is one
contiguous-per-partition DMA; each core computes its heads' attention
plus its partial output projection, and the host sums the 4 partials
per batch.

Device kernel (per core, everything resident in SBUF, matmul inputs in
fp16 with fp32 PSUM accumulation):
  xT [1024,2048] -> QT,KT [d,s] and V [s,d] projections, emitted as
  per-q-block "waves" interleaved into the attention stream.
  S^T tiles = matmul(lhsT=KT_blk, rhs=QT_blk): k on partitions, q on
  the free dim; head pairs target PE row groups 0-63/64-127
  back-to-back so their K=64 matmuls overlap in the PE array.
  exp on ScalarE (1/sqrt(D) folded into the activation scale); causal
  masking = never computing strictly-below-diagonal column ranges plus
  one 128x128 triangular mask multiply per diagonal block.
  P^T @ V with V augmented by a ones column (softmax denominator falls
  out of the same accumulation), software-pipelined one kb-step behind
  S/exp so the in-order PE queue never parks on an exp wait; the
  normalize chain (reciprocal + PE-broadcast of l) is deferred past the
  next row's S/exp for the same reason.  Y = O @ Wo streamed out per
  q-block (fp16) so output DMA overlaps remaining attention work.
  Startup: dummy warmup matmuls ramp the PE p-state while the DMA
  prefix (wq/wk head-pair-0 halves, x columns 0:512 per e-chunk, wv)
  lands; head-pair 0's QT/KT chains then run at full speed and the
  rest of wave 0 becomes attn(0) fill.  Projection/Y chains are split
  into ~2-matmul units and woven between attention kb-steps on
  independently paced fill streams to keep PE fed while ACT (exp)
  paces the softmax.
"""

import numpy as np
from contextlib import ExitStack

B, S, E, H, D = 2, 2048, 1024, 16, 64
N_CORES = 8
CPB = 4              # cores per batch
HL = H // CPB        # heads per core = 4
DL = HL * D          # local head dims = 256
P = 128              # partitions
EC = E // P          # 8 e-chunks
SB = S // P          # 16 s/k blocks
NQB = S // 512       # 4 q blocks of 512
MT = DL // P         # 2 row-tiles of QT/KT/OT (2 heads each)

_CACHE = {}
_EXHAUSTED = object()


def _emit(ctx, tc, xT, wq, wk, wv, wo, mask, ident, y, loop_n=0):
    import concourse.bass as bass  # noqa: F401
    from concourse import mybir

    nc = tc.nc
    f32 = mybir.dt.float32
    f16 = mybir.dt.float16
    Exp = mybir.ActivationFunctionType.Exp

    res = ctx.enter_context(tc.tile_pool(name="res", bufs=1))
    xt_sb = res.tile([P, EC, S], f16, tag="xt")
    wq_sb = res.tile([P, MT, EC, P], f16, tag="wq")
    wk_sb = res.tile([P, MT, EC, P], f16, tag="wk")
    wv_sb = res.tile([P, EC, DL], f16, tag="wv")
    wo_sb = res.tile([P, MT, E], f16, tag="wo")
    qt_sb = res.tile([P, MT, S], f16, tag="qt")
    kt_sb = res.tile([P, MT, S], f16, tag="kt")
    vt_sb = res.tile([P, SB, HL, D + 1], f16, tag="vt")
    ot_sb = res.tile([P, MT, S], f16, tag="ot")
    mask_sb = res.tile([P, P], f16, tag="mask")
    ident_sb = res.tile([P, P], f16, tag="ident")
    ones_sb = res.tile([P, D], f16, tag="ones")

    mm_ps = ctx.enter_context(tc.tile_pool(name="mm", bufs=2, space="PSUM"))
    s_ps = ctx.enter_context(tc.tile_pool(name="sps", bufs=2, space="PSUM"))
    o_ps = ctx.enter_context(tc.tile_pool(name="ops", bufs=2, space="PSUM"))

    e_pool = ctx.enter_context(tc.tile_pool(name="ep", bufs=34))
    y_pool = ctx.enter_context(tc.tile_pool(name="yp", bufs=6))
    l_pool = ctx.enter_context(tc.tile_pool(name="lp", bufs=6))

    def _full_body():
        dma = nc.sync

        nc.vector.memset(ones_sb[:], 1.0)
        nc.vector.memset(vt_sb[:, :, :, D:D + 1], 1.0)

        # ---- loads: critical prefix first.  Head-pair 0 needs wq/wk
        # halves plus x columns [0:512]; wv precedes them so the V(sb=0)
        # chain is never wv-gated.  All weights are host-swizzled so
        # each load is one contiguous DMA.  Everything else lands under
        # attn(0)/attn(1). ----
        HB = EC * P   # elements per wq/wk head-pair half
        dma.dma_start(out=wq_sb[:, 0], in_=wq[:, 0:HB])
        dma.dma_start(out=wk_sb[:, 0], in_=wk[:, 0:HB])
        for ec in range(EC):
            dma.dma_start(out=xt_sb[:, ec, 0:512],
                          in_=xT[ec * P:(ec + 1) * P, 0:512])
        dma.dma_start(out=wv_sb[:, 0:EC // 2], in_=wv[:, 0:EC * DL // 2])
        dma.dma_start(out=wv_sb[:, EC // 2:EC], in_=wv[:, EC * DL // 2:])
        dma.dma_start(out=mask_sb[:], in_=mask[:])
        dma.dma_start(out=ident_sb[:], in_=ident[:])
        dma.dma_start(out=wq_sb[:, 1], in_=wq[:, HB:2 * HB])
        dma.dma_start(out=wk_sb[:, 1], in_=wk[:, HB:2 * HB])
        xt_r = xT.rearrange("(ec p) s -> p ec s", p=P)
        dma.dma_start(out=xt_sb[:, :, 512:1024], in_=xt_r[:, :, 512:1024])
        dma.dma_start(out=xt_sb[:, :, 1024:1536], in_=xt_r[:, :, 1024:1536])
        dma.dma_start(out=wo_sb[:], in_=wo[:])
        dma.dma_start(out=xt_sb[:, :, 1536:2048], in_=xt_r[:, :, 1536:2048])

        # PE p-state warmup: ~3us of dummy matmuls (results discarded)
        # while the DMA prefix lands, so the real chains start at full
        # clock and the ramp never restarts.  Gated only on the ones
        # memset; sized to end right as the first x chunk is consumable.
        wp = o_ps.tile([P, 512], f32, tag="o")
        for _ in range(60):
            nc.tensor.matmul(wp[0:D, 0:D], ones_sb[0:1, :], ones_sb[0:1, :],
                             start=True, stop=True)

        def wave_units(nb, parts=("qt", "kt"), mts=range(MT), sbs=()):
            # QT/KT [:, mt, nb-window] = (w chunk)^T @ xT for mt in mts;
            # V[sb] for sb in sbs.  Generator yielding ~4-matmul units.
            srcs = []
            if "qt" in parts:
                srcs.append((wq_sb, qt_sb))
            if "kt" in parts:
                srcs.append((wk_sb, kt_sb))
            for mt in mts:
                for w_sb, t_sb in srcs:
                    ps = mm_ps.tile([P, 512], f32, tag="mm")
                    for ec in range(EC):
                        nc.tensor.matmul(
                            ps[:],
                            w_sb[:, mt, ec, :],
                            xt_sb[:, ec, nb * 512:(nb + 1) * 512],
                            start=(ec == 0), stop=(ec == EC - 1))
                        if ec == 3:
                            yield
                    nc.vector.tensor_copy(
                        t_sb[:, mt, nb * 512:(nb + 1) * 512], ps[:])
                    yield
            for sb in sbs:
                ps = mm_ps.tile([P, 512], f32, tag="mm")
                for ec in range(EC):
                    nc.tensor.matmul(
                        ps[:, 0:DL],
                        xt_sb[:, ec, sb * P:(sb + 1) * P],
                        wv_sb[:, ec, :],
                        start=(ec == 0), stop=(ec == EC - 1))
                    if ec == 3:
                        yield
                nc.vector.tensor_copy(
                    vt_sb[:, sb, :, 0:D],
                    ps[:, 0:DL].rearrange("p (h d) -> p h d", h=HL))
                yield

        def out_proj_units(qb, alt=False):
            # Y[sb, :] = O[sb, :] @ wo for this q-block's 4 s-blocks.
            # One merged DMA per s-block; with alt=True (the tail, where
            # ACT has no more exps) the two copies go to ACT/DVE so the
            # drain isn't serialized on one engine.
            Copy = mybir.ActivationFunctionType.Copy
            for sb in range(4 * qb, 4 * qb + 4):
                yt = y_pool.tile([P, E], f16, tag="y")
                tail_sb = alt and sb == 4 * qb + 3
                for eb in range(E // 512):
                    # tail: alternate PSUM pools so the copy-WAR pipeline
                    # is 4 deep and the drain stays PE-bound
                    pool = o_ps if alt and (2 * sb + eb) % 2 else mm_ps
                    yp = pool.tile([P, 512], f32,
                                   tag="o" if pool is o_ps else "mm")
                    for dc in range(MT):
                        nc.tensor.matmul(
                            yp[:],
                            ot_sb[:, dc, sb * P:(sb + 1) * P],
                            wo_sb[:, dc, eb * 512:(eb + 1) * 512],
                            start=(dc == 0), stop=(dc == MT - 1))
                    dst = yt[:, eb * 512:(eb + 1) * 512]
                    if alt and eb % 2 == 0:
                        nc.scalar.activation(out=dst, in_=yp[:], func=Copy)
                    else:
                        nc.vector.tensor_copy(dst, yp[:])
                    if tail_sb:
                        # last s-block: per-half DMA so the final transfer
                        # is small and starts as soon as its copy lands
                        dma.dma_start(
                            out=y[sb * P:(sb + 1) * P,
                                  eb * 512:(eb + 1) * 512],
                            in_=dst)
                    yield
                if not tail_sb:
                    dma.dma_start(out=y[sb * P:(sb + 1) * P, :], in_=yt[:])

        def fill_stream(gen, n, frac=1.0, t0=0):
            return {"gen": gen, "n": n, "done": 0, "frac": frac, "t0": t0}

        pending_norm = []

        def make_norm(qb, mt, obs):
            # normalize + reorient: O sits q-on-partitions as [q, qc, d+1]
            # (ob = SBUF copy that freed the PSUM bank at row end), so the
            # softmax denominator divide is a per-partition scalar multiply;
            # PE then transposes each normalized [q,d] chunk back to [d,q]
            # for the output projection.  All of this runs long after the
            # ob copy, so the PE queue never parks here.
            def emit():
                for half in range(2):
                    ob = obs[half]
                    dr = half * D
                    rec = l_pool.tile([P, 4, 1], f32, tag="rec")
                    with nc.allow_low_precision(
                            reason="1/l in fp16 adds ~5e-4 rel err; "
                                   "budget is 2e-2"):
                        nc.vector.reciprocal(rec[:], ob[:, :, D:D + 1])
                    on = l_pool.tile([P, 4, D], f16, tag="on")
                    for qc in range(4):
                        nc.vector.tensor_scalar_mul(
                            on[:, qc, :], ob[:, qc, 0:D], rec[:, qc, :])
                    tr = o_ps.tile([D, 4, P], f16, tag="o")
                    for qc in range(4):
                        nc.tensor.transpose(tr[:, qc, :], on[:, qc, :],
                                            ident_sb[:])
                    nc.vector.tensor_copy(
                        ot_sb[dr:dr + D, mt, qb * 512:(qb + 1) * 512],
                        tr[:].rearrange("d qc p -> d (qc p)"))
            return emit

        pending_pv = []

        def flush_norm():
            while pending_norm:
                pending_norm.pop(0)()

        def attention_block(qb, streams):
            # ACT-paced; PV runs one kb-step behind S/exp so the in-order
            # PE queue always has ready work (S of step k+1) ahead of the
            # PV that waits on exp(k).  Fill streams are paced
            # independently: waves front-loaded (frac<1) to meet their
            # consumers, Y chains spread across the whole block.
            nkb = 4 * (qb + 1)     # causal: k blocks 0 .. nkb-1
            scale = float(1.0 / np.sqrt(D))
            nsteps = MT * nkb
            step = 0

            def run_fill():
                for st in streams:
                    fsteps = max(1, int((nsteps - st["t0"]) * st["frac"]))
                    want = min(st["n"],
                               ((step + 1 - st["t0"]) * st["n"]) // fsteps)
                    while st["done"] < want:
                        if next(st["gen"], _EXHAUSTED) is _EXHAUSTED:
                            st["done"] = st["n"]
                            break
                        st["done"] += 1

            def make_pv(mt, ets, last):
                # Deferred O[q, d] burst: lhsT = 128x128 chunks of P^T
                # (stationary), rhs = V (+ones column) -> 65-row matmuls.
                # Chunk-major so each q-chunk's PSUM accumulation is one
                # CONTIGUOUS group: interleaved open groups in a bank
                # corrupt each other (start resets the bank context).
                # Consumed as a paced fill stream over the following row
                # / block, covering its ACT-paced S/exp phase.  Only one
                # burst is ever in flight, so the o-pool rotation stays
                # sequential (oa,oa -> tr,tr per row-slot).
                def gen():
                    oaccs = []
                    for _h in range(2):
                        oa = o_ps.tile([P, 4, D + 1], f32, tag="o",
                                       name=f"oa{_h}")
                        oaccs.append(oa)
                    cnt = 0
                    for half in range(2):
                        for qc in range(4):
                            for kb in range(4 * qb + qc + 1):
                                nc.tensor.matmul(
                                    oaccs[half][:, qc, :],
                                    ets[kb][:, half * 512 + qc * P:
                                            half * 512 + (qc + 1) * P],
                                    vt_sb[:, kb, 2 * mt + half, :],
                                    start=(kb == 0),
                                    stop=(kb == 4 * qb + qc))
                                cnt += 1
                                if cnt % 6 == 0:
                                    yield
                    obs = []
                    for half in range(2):
                        ob = l_pool.tile([P, 4, D + 1], f16, tag="ob",
                                         name=f"ob{half}")
                        if last and half == 1:
                            nc.scalar.activation(
                                out=ob[:], in_=oaccs[half][:],
                                func=mybir.ActivationFunctionType.Copy)
                        else:
                            nc.vector.tensor_copy(ob[:], oaccs[half][:])
                        obs.append(ob)
                    pending_norm.append(make_norm(qb, mt, obs))
                n = 2 * sum(4 * qb + qc + 1 for qc in range(4)) // 6 + 1
                return gen(), n

            for mt in range(MT):   # head pair (2*mt, 2*mt+1)
                ets = []
                for kb in range(nkb):
                    t = kb - 4 * qb
                    v0 = P * t if t > 0 else 0   # masked prefix of window
                    sp = s_ps.tile([P, 1024], f32, tag="s")
                    for half in range(2):
                        dr = half * D
                        nc.tensor.matmul(
                            sp[:, half * 512 + v0:(half + 1) * 512],
                            kt_sb[dr:dr + D, mt, kb * P:(kb + 1) * P],
                            qt_sb[dr:dr + D, mt,
                                  qb * 512 + v0:(qb + 1) * 512],
                            start=True, stop=True)
                    et = e_pool.tile([P, 1024], f16, tag="e")
                    nc.scalar.activation(out=et[:, v0:], in_=sp[:, v0:],
                                         func=Exp, scale=scale)
                    if t >= 0:  # diagonal block: mask strictly-future keys
                        for half in range(2):
                            w0 = half * 512 + v0
                            nc.vector.tensor_mul(
                                et[:, w0:w0 + P], et[:, w0:w0 + P],
                                mask_sb[:])
                    if kb == 0:
                        # finish any unfinished previous burst first (two
                        # open bursts would interleave on the o-pool and
                        # corrupt each other's PSUM groups), then hand the
                        # previous row's burst to the fill scheduler and
                        # flush the one-older normalize
                        for st in streams:
                            if st.get("pv"):
                                while (next(st["gen"], _EXHAUSTED)
                                       is not _EXHAUSTED):
                                    pass
                                st["done"] = st["n"]
                        while pending_pv:
                            g, n = pending_pv.pop(0)
                            st = fill_stream(g, n, 0.85, t0=step)
                            st["pv"] = True
                            streams.append(st)
                        flush_norm()
                    run_fill()
                    step += 1
                    ets.append(et)
                if mt == MT - 1:
                    # drain leftover fill (including the mt0 burst) so
                    # nothing is left queued behind the trailing exps
                    for st in streams:
                        while next(st["gen"], _EXHAUSTED) is not _EXHAUSTED:
                            pass
                last = qb == NQB - 1 and mt == MT - 1
                pending_pv.append(make_pv(mt, ets, last))

        # startup: head-pair 0's qt/kt chains interleaved per e-chunk
        # (paced by the arriving x column-block DMAs), then V(sb=0).
        pq = mm_ps.tile([P, 512], f32, tag="mm")
        pk = o_ps.tile([P, 512], f32, tag="o")
        for ec in range(EC):
            nc.tensor.matmul(pq[:], wq_sb[:, 0, ec, :],
                             xt_sb[:, ec, 0:512],
                             start=(ec == 0), stop=(ec == EC - 1))
            nc.tensor.matmul(pk[:], wk_sb[:, 0, ec, :],
                             xt_sb[:, ec, 0:512],
                             start=(ec == 0), stop=(ec == EC - 1))
        nc.vector.tensor_copy(qt_sb[:, 0, 0:512], pq[:])
        nc.vector.tensor_copy(kt_sb[:, 0, 0:512], pk[:])
        for _ in wave_units(0, parts=(), sbs=(0,)):
            pass

        def _chain(*gens):
            for g in gens:
                yield from g

        # Fill plan: attn(0) finishes wave 0 (V sb1-3 + head-pair 1) and
        # runs all of wave(1); attn(1) takes wave(2) + Y(0); attn(2)
        # takes wave(3)'s qt/kt; attn(3) takes wave(3)'s V chains
        # (front-loaded to land before the kb=12 diagonal) + Y(1) + Y(2).
        plans = [
            [fill_stream(_chain(wave_units(0, parts=(), sbs=(1, 2, 3)),
                                wave_units(0, mts=[1]),
                                wave_units(1, sbs=(4, 5, 6, 7))), 34, 1.0)],
            [fill_stream(wave_units(2, sbs=(8, 9, 10, 11)), 24, 0.95)],
            [fill_stream(wave_units(3), 8, 0.9),
             fill_stream(out_proj_units(0), 8, 0.95)],
            [fill_stream(wave_units(3, parts=(), sbs=(12, 13, 14, 15)),
                         8, 0.5),
             fill_stream(_chain(out_proj_units(1), out_proj_units(2)),
                         16, 1.15)],
        ]
        for qb in range(NQB):
            attention_block(qb, plans[qb])
        while pending_pv:   # attn(3) mt1's burst: overlaps trailing exps
            g, _n = pending_pv.pop(0)
            for _ in g:
                pass
        flush_norm()
        for _ in out_proj_units(NQB - 1, alt=True):
            pass

    if loop_n:
        # bench-only path: hint all engines so the back-edge prefetches
        # the body's IRAM blocks (body >256 instructions per engine)
        from concourse import mybir
        hints = (mybir.EngineType.PE, mybir.EngineType.Activation,
                 mybir.EngineType.DVE, mybir.EngineType.SP,
                 mybir.EngineType.Pool)
        with tc.For_i(0, loop_n, 1, hint_engines=hints):
            _full_body()
    else:
        _full_body()


def _get_program(loop_n=0):
    key = ("nc", loop_n)
    if key in _CACHE:
        return _CACHE[key]
    import concourse.tile as tile
    from concourse import bacc, mybir

    f16 = mybir.dt.float16
    nc = bacc.Bacc("TRN2", target_bir_lowering=False, debug=False,
                   enable_asserts=False)
    xT = nc.dram_tensor("xT", [E, S], f16, kind="ExternalInput").ap()
    wq = nc.dram_tensor("wq", [P, MT * EC * P], f16, kind="ExternalInput").ap()
    wk = nc.dram_tensor("wk", [P, MT * EC * P], f16, kind="ExternalInput").ap()
    wv = nc.dram_tensor("wv", [P, EC * DL], f16, kind="ExternalInput").ap()
    wo = nc.dram_tensor("wo", [P, MT * E], f16, kind="ExternalInput").ap()
    mask = nc.dram_tensor("mask", [P, P], f16, kind="ExternalInput").ap()
    ident = nc.dram_tensor("ident", [P, P], f16, kind="ExternalInput").ap()
    y = nc.dram_tensor("y", [S, E], f16, kind="ExternalOutput").ap()
    with tile.TileContext(nc) as tc:
        with ExitStack() as ctx:
            _emit(ctx, tc, xT, wq, wk, wv, wo, mask, ident, y, loop_n=loop_n)
    nc.compile()
    _CACHE[key] = nc
    return nc


def _make_in_maps(x, Wq, Wk, Wv, Wo):
    x = np.asarray(x, dtype=np.float32)
    Wq = np.asarray(Wq, dtype=np.float32)
    Wk = np.asarray(Wk, dtype=np.float32)
    Wv = np.asarray(Wv, dtype=np.float32)
    Wo = np.asarray(Wo, dtype=np.float32)
    mask = np.triu(np.ones((P, P), dtype=np.float16))
    in_maps = []
    for c in range(N_CORES):
        b, hg = divmod(c, CPB)
        hs = slice(hg * HL, (hg + 1) * HL)
        # per-core slices: columns (heads) of Wq/Wk/Wv, rows of Wo
        wq_c = Wq.reshape(E, H, D)[:, hs, :].reshape(E, DL)
        wk_c = Wk.reshape(E, H, D)[:, hs, :].reshape(E, DL)
        wv_c = Wv.reshape(E, H, D)[:, hs, :].reshape(E, DL)
        wo_c = Wo.reshape(H, D, E)[hs, :, :].reshape(DL, E)
        # swizzle into device SBUF layouts (contiguous per partition):
        #   wq/wk: [P, MT, EC, P],  wv: [P, EC, DL],  wo: [P, MT, E]
        wq_c = wq_c.reshape(EC, P, MT, P).transpose(1, 2, 0, 3).reshape(P, -1)
        wk_c = wk_c.reshape(EC, P, MT, P).transpose(1, 2, 0, 3).reshape(P, -1)
        wv_c = wv_c.reshape(EC, P, DL).transpose(1, 0, 2).reshape(P, -1)
        wo_c = wo_c.reshape(MT, P, E).transpose(1, 0, 2).reshape(P, -1)
        in_maps.append({
            "xT": np.ascontiguousarray(x[b].T).astype(np.float16),
            "wq": np.ascontiguousarray(wq_c).astype(np.float16),
            "wk": np.ascontiguousarray(wk_c).astype(np.float16),
            "wv": np.ascontiguousarray(wv_c).astype(np.float16),
            "wo": np.ascontiguousarray(wo_c).astype(np.float16),
            "mask": mask,
            "ident": np.eye(P, dtype=np.float16),
        })
    return in_maps


def run(x, Wq, Wk, Wv, Wo, trace=False):
    from concourse.bass_utils import run_bass_kernel_spmd

    nc = _get_program()
    in_maps = _make_in_maps(x, Wq, Wk, Wv, Wo)
    br = run_bass_kernel_spmd(nc, in_maps, list(range(N_CORES)), trace=trace)
    out = np.zeros((B, S, E), dtype=np.float32)
    for c in range(N_CORES):
        out[c // CPB] += br.results[c]["y"]
    return out, br


def kernel(x, Wq, Wk, Wv, Wo):
    out, _ = run(x, Wq, Wk, Wv, Wo, trace=False)
    return out


# revision 65
# speedup vs baseline: 1.1042x; 1.1042x over previous
"""Causal self-attention (B=2, S=2048, E=1024, H=16) on 8 Trainium2 cores.

Sharding: core c in 0..7 handles batch b = c//4 and the 4 heads
[4*(c%4), 4*(c%4)+4).  The host pre-transposes x[b], pre-slices the QKV
weights column-wise / Wo row-wise per core, and pre-swizzles each
weight into the device SBUF layout so every weight load is one
contiguous-per-partition DMA; each core computes its heads' attention
plus its partial output projection, and the host sums the 4 partials
per batch.

Device kernel (per core, everything resident in SBUF, matmul inputs in
fp16 with fp32 PSUM accumulation):
  xT [1024,2048] -> QT,KT [d,s] and V [s,d] projections, emitted as
  per-q-block "waves" interleaved into the attention stream.
  S^T tiles = matmul(lhsT=KT_blk, rhs=QT_blk): k on partitions, q on
  the free dim; head pairs target PE row groups 0-63/64-127
  back-to-back so their K=64 matmuls overlap in the PE array.
  exp on ScalarE (1/sqrt(D) folded into the activation scale); causal
  masking = never computing strictly-below-diagonal column ranges plus
  one 128x128 triangular mask multiply per diagonal block.
  The attention row keeps its exp'd tiles resident and defers P^T @ V
  into a chunk-major "burst": out O[q-chunk, d] with lhsT = 128x128
  chunks of P^T and rhs = V augmented by a ones column (the softmax
  denominator falls out of the same 65-row matmuls, and each q-chunk's
  causal k-range is exact).  Each chunk's PSUM accumulation is one
  contiguous group (interleaved open groups corrupt a bank), and the
  burst is consumed as a paced fill stream over the following row, so
  the in-order PE queue never parks on exp waits.  With O q-major the
  softmax divide is a per-partition tensor_scalar multiply (no
  broadcast matmul); PE transposes (via a host identity) restore O^T
  for Y = O @ Wo, streamed out per s-block (fp16) so output DMA
  overlaps remaining attention work.
  Startup: dummy warmup matmuls ramp the PE p-state while the DMA
  prefix (wq/wk head-pair-0 halves, x columns 0:512 per e-chunk, wv)
  lands; head-pair 0's QT/KT chains then run at full speed and the
  rest of wave 0 becomes attn(0) fill.  Projection/Y chains are split
  into ~2-matmul units and woven between attention kb-steps on
  independently paced fill streams to keep PE fed while ACT (exp)
  paces the softmax.
"""

import numpy as np
from contextlib import ExitStack

B, S, E, H, D = 2, 2048, 1024, 16, 64
N_CORES = 8
CPB = 4              # cores per batch
HL = H // CPB        # heads per core = 4
DL = HL * D          # local head dims = 256
P = 128              # partitions
EC = E // P          # 8 e-chunks
SB = S // P          # 16 s/k blocks
NQB = S // 512       # 4 q blocks of 512
MT = DL // P         # 2 row-tiles of QT/KT/OT (2 heads each)

_CACHE = {}
_EXHAUSTED = object()


def _emit(ctx, tc, xT, wq, wk, wv, wo, mask, ident, y, loop_n=0):
    import concourse.bass as bass  # noqa: F401
    from concourse import mybir

    nc = tc.nc
    f32 = mybir.dt.float32
    f16 = mybir.dt.float16
    Exp = mybir.ActivationFunctionType.Exp

    res = ctx.enter_context(tc.tile_pool(name="res", bufs=1))
    xt_sb = res.tile([P, EC, S], f16, tag="xt")
    wq_sb = res.tile([P, MT, EC, P], f16, tag="wq")
    wk_sb = res.tile([P, MT, EC, P], f16, tag="wk")
    wv_sb = res.tile([P, EC, DL], f16, tag="wv")
    wo_sb = res.tile([P, MT, E], f16, tag="wo")
    qt_sb = res.tile([P, MT, S], f16, tag="qt")
    kt_sb = res.tile([P, MT, S], f16, tag="kt")
    vt_sb = res.tile([P, SB, HL, D + 1], f16, tag="vt")
    ot_sb = res.tile([P, MT, S], f16, tag="ot")
    mask_sb = res.tile([P, P], f16, tag="mask")
    ident_sb = res.tile([P, P], f16, tag="ident")
    ones_sb = res.tile([P, D], f16, tag="ones")

    mm_ps = ctx.enter_context(tc.tile_pool(name="mm", bufs=2, space="PSUM"))
    s_ps = ctx.enter_context(tc.tile_pool(name="sps", bufs=2, space="PSUM"))
    o_ps = ctx.enter_context(tc.tile_pool(name="ops", bufs=2, space="PSUM"))

    e_pool = ctx.enter_context(tc.tile_pool(name="ep", bufs=34))
    y_pool = ctx.enter_context(tc.tile_pool(name="yp", bufs=6))
    l_pool = ctx.enter_context(tc.tile_pool(name="lp", bufs=6))

    def _full_body():
        dma = nc.sync

        nc.vector.memset(ones_sb[:], 1.0)
        nc.vector.memset(vt_sb[:, :, :, D:D + 1], 1.0)

        # ---- loads: critical prefix first.  Head-pair 0 needs wq/wk
        # halves plus x columns [0:512]; wv precedes them so the V(sb=0)
        # chain is never wv-gated.  All weights are host-swizzled so
        # each load is one contiguous DMA.  Everything else lands under
        # attn(0)/attn(1). ----
        HB = EC * P   # elements per wq/wk head-pair half
        dma.dma_start(out=wq_sb[:, 0], in_=wq[:, 0:HB])
        dma.dma_start(out=wk_sb[:, 0], in_=wk[:, 0:HB])
        for ec in range(EC):
            dma.dma_start(out=xt_sb[:, ec, 0:512],
                          in_=xT[ec * P:(ec + 1) * P, 0:512])
        dma.dma_start(out=wv_sb[:, 0:EC // 2], in_=wv[:, 0:EC * DL // 2])
        dma.dma_start(out=wv_sb[:, EC // 2:EC], in_=wv[:, EC * DL // 2:])
        dma.dma_start(out=mask_sb[:], in_=mask[:])
        dma.dma_start(out=ident_sb[:], in_=ident[:])
        dma.dma_start(out=wq_sb[:, 1], in_=wq[:, HB:2 * HB])
        dma.dma_start(out=wk_sb[:, 1], in_=wk[:, HB:2 * HB])
        xt_r = xT.rearrange("(ec p) s -> p ec s", p=P)
        dma.dma_start(out=xt_sb[:, :, 512:1024], in_=xt_r[:, :, 512:1024])
        dma.dma_start(out=xt_sb[:, :, 1024:1536], in_=xt_r[:, :, 1024:1536])
        dma.dma_start(out=wo_sb[:], in_=wo[:])
        dma.dma_start(out=xt_sb[:, :, 1536:2048], in_=xt_r[:, :, 1536:2048])

        # PE p-state warmup: ~3us of dummy matmuls (results discarded)
        # while the DMA prefix lands, so the real chains start at full
        # clock and the ramp never restarts.  Gated only on the ones
        # memset; sized to end right as the first x chunk is consumable.
        wp = o_ps.tile([P, 512], f32, tag="o")
        for _ in range(60):
            nc.tensor.matmul(wp[0:D, 0:D], ones_sb[0:1, :], ones_sb[0:1, :],
                             start=True, stop=True)

        def wave_units(nb, parts=("qt", "kt"), mts=range(MT), sbs=()):
            # QT/KT [:, mt, nb-window] = (w chunk)^T @ xT for mt in mts;
            # V[sb] for sb in sbs.  Generator yielding ~4-matmul units.
            srcs = []
            if "qt" in parts:
                srcs.append((wq_sb, qt_sb))
            if "kt" in parts:
                srcs.append((wk_sb, kt_sb))
            for mt in mts:
                for w_sb, t_sb in srcs:
                    ps = mm_ps.tile([P, 512], f32, tag="mm")
                    for ec in range(EC):
                        nc.tensor.matmul(
                            ps[:],
                            w_sb[:, mt, ec, :],
                            xt_sb[:, ec, nb * 512:(nb + 1) * 512],
                            start=(ec == 0), stop=(ec == EC - 1))
                        if ec == 3:
                            yield
                    nc.vector.tensor_copy(
                        t_sb[:, mt, nb * 512:(nb + 1) * 512], ps[:])
                    yield
            for sb in sbs:
                ps = mm_ps.tile([P, 512], f32, tag="mm")
                for ec in range(EC):
                    nc.tensor.matmul(
                        ps[:, 0:DL],
                        xt_sb[:, ec, sb * P:(sb + 1) * P],
                        wv_sb[:, ec, :],
                        start=(ec == 0), stop=(ec == EC - 1))
                    if ec == 3:
                        yield
                nc.vector.tensor_copy(
                    vt_sb[:, sb, :, 0:D],
                    ps[:, 0:DL].rearrange("p (h d) -> p h d", h=HL))
                yield

        def out_proj_units(qb, alt=False):
            # Y[sb, :] = O[sb, :] @ wo for this q-block's 4 s-blocks.
            # One merged DMA per s-block; with alt=True (the tail, where
            # ACT has no more exps) the two copies go to ACT/DVE so the
            # drain isn't serialized on one engine.
            Copy = mybir.ActivationFunctionType.Copy
            for sb in range(4 * qb, 4 * qb + 4):
                yt = y_pool.tile([P, E], f16, tag="y")
                tail_sb = alt and sb == 4 * qb + 3
                for eb in range(E // 512):
                    # tail: alternate PSUM pools so the copy-WAR pipeline
                    # is 4 deep and the drain stays PE-bound
                    pool = o_ps if alt and (2 * sb + eb) % 2 else mm_ps
                    yp = pool.tile([P, 512], f32,
                                   tag="o" if pool is o_ps else "mm")
                    for dc in range(MT):
                        nc.tensor.matmul(
                            yp[:],
                            ot_sb[:, dc, sb * P:(sb + 1) * P],
                            wo_sb[:, dc, eb * 512:(eb + 1) * 512],
                            start=(dc == 0), stop=(dc == MT - 1))
                    dst = yt[:, eb * 512:(eb + 1) * 512]
                    if alt and eb % 2 == 0:
                        nc.scalar.activation(out=dst, in_=yp[:], func=Copy)
                    else:
                        nc.vector.tensor_copy(dst, yp[:])
                    if tail_sb:
                        # last s-block: per-half DMA so the final transfer
                        # is small and starts as soon as its copy lands
                        dma.dma_start(
                            out=y[sb * P:(sb + 1) * P,
                                  eb * 512:(eb + 1) * 512],
                            in_=dst)
                    yield
                if not tail_sb:
                    dma.dma_start(out=y[sb * P:(sb + 1) * P, :], in_=yt[:])

        def fill_stream(gen, n, frac=1.0, t0=0):
            return {"gen": gen, "n": n, "done": 0, "frac": frac, "t0": t0}

        pending_norm = []

        def make_norm(qb, mt, obs):
            # normalize + reorient: O sits q-on-partitions as [q, qc, d+1]
            # (ob = SBUF copy that freed the PSUM bank at row end), so the
            # softmax denominator divide is a per-partition scalar multiply;
            # PE then transposes each normalized [q,d] chunk back to [d,q]
            # for the output projection.  All of this runs long after the
            # ob copy, so the PE queue never parks here.
            def emit():
                for half in range(2):
                    ob = obs[half]
                    dr = half * D
                    rec = l_pool.tile([P, 4, 1], f32, tag="rec")
                    with nc.allow_low_precision(
                            reason="1/l in fp16 adds ~5e-4 rel err; "
                                   "budget is 2e-2"):
                        nc.vector.reciprocal(rec[:], ob[:, :, D:D + 1])
                    on = l_pool.tile([P, 4, D], f16, tag="on")
                    for qc in range(4):
                        nc.vector.tensor_scalar_mul(
                            on[:, qc, :], ob[:, qc, 0:D], rec[:, qc, :])
                    tr = o_ps.tile([D, 4, P], f16, tag="o")
                    for qc in range(4):
                        nc.tensor.transpose(tr[:, qc, :], on[:, qc, :],
                                            ident_sb[:])
                    nc.vector.tensor_copy(
                        ot_sb[dr:dr + D, mt, qb * 512:(qb + 1) * 512],
                        tr[:].rearrange("d qc p -> d (qc p)"))
            return emit

        pending_pv = []

        def flush_norm():
            while pending_norm:
                pending_norm.pop(0)()

        def attention_block(qb, streams):
            # ACT-paced; PV runs one kb-step behind S/exp so the in-order
            # PE queue always has ready work (S of step k+1) ahead of the
            # PV that waits on exp(k).  Fill streams are paced
            # independently: waves front-loaded (frac<1) to meet their
            # consumers, Y chains spread across the whole block.
            nkb = 4 * (qb + 1)     # causal: k blocks 0 .. nkb-1
            scale = float(1.0 / np.sqrt(D))
            nsteps = MT * nkb
            step = 0

            def run_fill():
                for st in streams:
                    fsteps = max(1, int((nsteps - st["t0"]) * st["frac"]))
                    want = min(st["n"],
                               ((step + 1 - st["t0"]) * st["n"]) // fsteps)
                    while st["done"] < want:
                        if next(st["gen"], _EXHAUSTED) is _EXHAUSTED:
                            st["done"] = st["n"]
                            break
                        st["done"] += 1

            def make_pv(mt, ets, last):
                # Deferred O[q, d] burst: lhsT = 128x128 chunks of P^T
                # (stationary), rhs = V (+ones column) -> 65-row matmuls.
                # Chunk-major so each q-chunk's PSUM accumulation is one
                # CONTIGUOUS group: interleaved open groups in a bank
                # corrupt each other (start resets the bank context).
                # Consumed as a paced fill stream over the following row
                # / block, covering its ACT-paced S/exp phase.  Only one
                # burst is ever in flight, so the o-pool rotation stays
                # sequential (oa,oa -> tr,tr per row-slot).
                def gen():
                    oaccs = []
                    for _h in range(2):
                        oa = o_ps.tile([P, 4, D + 1], f32, tag="o",
                                       name=f"oa{_h}")
                        oaccs.append(oa)
                    cnt = 0
                    for half in range(2):
                        for qc in range(4):
                            for kb in range(4 * qb + qc + 1):
                                nc.tensor.matmul(
                                    oaccs[half][:, qc, :],
                                    ets[kb][:, half * 512 + qc * P:
                                            half * 512 + (qc + 1) * P],
                                    vt_sb[:, kb, 2 * mt + half, :],
                                    start=(kb == 0),
                                    stop=(kb == 4 * qb + qc))
                                cnt += 1
                                if cnt % 6 == 0:
                                    yield
                    obs = []
                    for half in range(2):
                        ob = l_pool.tile([P, 4, D + 1], f16, tag="ob",
                                         name=f"ob{half}")
                        if last and half == 1:
                            nc.scalar.activation(
                                out=ob[:], in_=oaccs[half][:],
                                func=mybir.ActivationFunctionType.Copy)
                        else:
                            nc.vector.tensor_copy(ob[:], oaccs[half][:])
                        obs.append(ob)
                    pending_norm.append(make_norm(qb, mt, obs))
                n = 2 * sum(4 * qb + qc + 1 for qc in range(4)) // 6 + 1
                return gen(), n

            for mt in range(MT):   # head pair (2*mt, 2*mt+1)
                ets = []
                for kb in range(nkb):
                    t = kb - 4 * qb
                    v0 = P * t if t > 0 else 0   # masked prefix of window
                    sp = s_ps.tile([P, 1024], f32, tag="s")
                    for half in range(2):
                        dr = half * D
                        nc.tensor.matmul(
                            sp[:, half * 512 + v0:(half + 1) * 512],
                            kt_sb[dr:dr + D, mt, kb * P:(kb + 1) * P],
                            qt_sb[dr:dr + D, mt,
                                  qb * 512 + v0:(qb + 1) * 512],
                            start=True, stop=True)
                    et = e_pool.tile([P, 1024], f16, tag="e")
                    nc.scalar.activation(out=et[:, v0:], in_=sp[:, v0:],
                                         func=Exp, scale=scale)
                    if t >= 0:  # diagonal block: mask strictly-future keys
                        for half in range(2):
                            w0 = half * 512 + v0
                            nc.vector.tensor_mul(
                                et[:, w0:w0 + P], et[:, w0:w0 + P],
                                mask_sb[:])
                    if kb == 0:
                        # finish any unfinished previous burst first (two
                        # open bursts would interleave on the o-pool and
                        # corrupt each other's PSUM groups), then hand the
                        # previous row's burst to the fill scheduler and
                        # flush the one-older normalize
                        for st in streams:
                            if st.get("pv"):
                                while (next(st["gen"], _EXHAUSTED)
                                       is not _EXHAUSTED):
                                    pass
                                st["done"] = st["n"]
                        while pending_pv:
                            g, n = pending_pv.pop(0)
                            st = fill_stream(g, n, 0.85, t0=step)
                            st["pv"] = True
                            streams.append(st)
                        flush_norm()
                    run_fill()
                    step += 1
                    ets.append(et)
                if mt == MT - 1:
                    # drain leftover fill (including the mt0 burst) so
                    # nothing is left queued behind the trailing exps
                    for st in streams:
                        while next(st["gen"], _EXHAUSTED) is not _EXHAUSTED:
                            pass
                last = qb == NQB - 1 and mt == MT - 1
                pending_pv.append(make_pv(mt, ets, last))

        # startup: head-pair 0's qt/kt chains interleaved per e-chunk
        # (paced by the arriving x column-block DMAs), then V(sb=0).
        pq = mm_ps.tile([P, 512], f32, tag="mm")
        pk = o_ps.tile([P, 512], f32, tag="o")
        for ec in range(EC):
            nc.tensor.matmul(pq[:], wq_sb[:, 0, ec, :],
                             xt_sb[:, ec, 0:512],
                             start=(ec == 0), stop=(ec == EC - 1))
            nc.tensor.matmul(pk[:], wk_sb[:, 0, ec, :],
                             xt_sb[:, ec, 0:512],
                             start=(ec == 0), stop=(ec == EC - 1))
        nc.vector.tensor_copy(qt_sb[:, 0, 0:512], pq[:])
        nc.vector.tensor_copy(kt_sb[:, 0, 0:512], pk[:])
        for _ in wave_units(0, parts=(), sbs=(0,)):
            pass

        def _chain(*gens):
            for g in gens:
                yield from g

        # Fill plan: attn(0) finishes wave 0 (V sb1-3 + head-pair 1) and
        # runs all of wave(1); attn(1) takes wave(2) + Y(0); attn(2)
        # takes wave(3)'s qt/kt; attn(3) takes wave(3)'s V chains
        # (front-loaded to land before the kb=12 diagonal) + Y(1) + Y(2).
        plans = [
            [fill_stream(_chain(wave_units(0, parts=(), sbs=(1, 2, 3)),
                                wave_units(0, mts=[1]),
                                wave_units(1, sbs=(4, 5, 6, 7))), 34, 1.0)],
            [fill_stream(wave_units(2, sbs=(8, 9, 10, 11)), 24, 0.95)],
            [fill_stream(wave_units(3), 8, 0.9),
             fill_stream(out_proj_units(0), 8, 0.95)],
            [fill_stream(wave_units(3, parts=(), sbs=(12, 13, 14, 15)),
                         8, 0.5),
             fill_stream(_chain(out_proj_units(1), out_proj_units(2)),
                         16, 1.15)],
        ]
        for qb in range(NQB):
            attention_block(qb, plans[qb])
        while pending_pv:   # attn(3) mt1's burst: overlaps trailing exps
            g, _n = pending_pv.pop(0)
            for _ in g:
                pass
        flush_norm()
        for _ in out_proj_units(NQB - 1, alt=True):
            pass

    if loop_n:
        # bench-only path: hint all engines so the back-edge prefetches
        # the body's IRAM blocks (body >256 instructions per engine)
        from concourse import mybir
        hints = (mybir.EngineType.PE, mybir.EngineType.Activation,
                 mybir.EngineType.DVE, mybir.EngineType.SP,
                 mybir.EngineType.Pool)
        with tc.For_i(0, loop_n, 1, hint_engines=hints):
            _full_body()
    else:
        _full_body()


def _get_program(loop_n=0):
    key = ("nc", loop_n)
    if key in _CACHE:
        return _CACHE[key]
    import concourse.tile as tile
    from concourse import bacc, mybir

    f16 = mybir.dt.float16
    nc = bacc.Bacc("TRN2", target_bir_lowering=False, debug=False,
                   enable_asserts=False)
    xT = nc.dram_tensor("xT", [E, S], f16, kind="ExternalInput").ap()
    wq = nc.dram_tensor("wq", [P, MT * EC * P], f16, kind="ExternalInput").ap()
    wk = nc.dram_tensor("wk", [P, MT * EC * P], f16, kind="ExternalInput").ap()
    wv = nc.dram_tensor("wv", [P, EC * DL], f16, kind="ExternalInput").ap()
    wo = nc.dram_tensor("wo", [P, MT * E], f16, kind="ExternalInput").ap()
    mask = nc.dram_tensor("mask", [P, P], f16, kind="ExternalInput").ap()
    ident = nc.dram_tensor("ident", [P, P], f16, kind="ExternalInput").ap()
    y = nc.dram_tensor("y", [S, E], f16, kind="ExternalOutput").ap()
    with tile.TileContext(nc) as tc:
        with ExitStack() as ctx:
            _emit(ctx, tc, xT, wq, wk, wv, wo, mask, ident, y, loop_n=loop_n)
    nc.compile()
    _CACHE[key] = nc
    return nc


def _make_in_maps(x, Wq, Wk, Wv, Wo):
    x = np.asarray(x, dtype=np.float32)
    Wq = np.asarray(Wq, dtype=np.float32)
    Wk = np.asarray(Wk, dtype=np.float32)
    Wv = np.asarray(Wv, dtype=np.float32)
    Wo = np.asarray(Wo, dtype=np.float32)
    mask = np.triu(np.ones((P, P), dtype=np.float16))
    in_maps = []
    for c in range(N_CORES):
        b, hg = divmod(c, CPB)
        hs = slice(hg * HL, (hg + 1) * HL)
        # per-core slices: columns (heads) of Wq/Wk/Wv, rows of Wo
        wq_c = Wq.reshape(E, H, D)[:, hs, :].reshape(E, DL)
        wk_c = Wk.reshape(E, H, D)[:, hs, :].reshape(E, DL)
        wv_c = Wv.reshape(E, H, D)[:, hs, :].reshape(E, DL)
        wo_c = Wo.reshape(H, D, E)[hs, :, :].reshape(DL, E)
        # swizzle into device SBUF layouts (contiguous per partition):
        #   wq/wk: [P, MT, EC, P],  wv: [P, EC, DL],  wo: [P, MT, E]
        wq_c = wq_c.reshape(EC, P, MT, P).transpose(1, 2, 0, 3).reshape(P, -1)
        wk_c = wk_c.reshape(EC, P, MT, P).transpose(1, 2, 0, 3).reshape(P, -1)
        wv_c = wv_c.reshape(EC, P, DL).transpose(1, 0, 2).reshape(P, -1)
        wo_c = wo_c.reshape(MT, P, E).transpose(1, 0, 2).reshape(P, -1)
        in_maps.append({
            "xT": np.ascontiguousarray(x[b].T).astype(np.float16),
            "wq": np.ascontiguousarray(wq_c).astype(np.float16),
            "wk": np.ascontiguousarray(wk_c).astype(np.float16),
            "wv": np.ascontiguousarray(wv_c).astype(np.float16),
            "wo": np.ascontiguousarray(wo_c).astype(np.float16),
            "mask": mask,
            "ident": np.eye(P, dtype=np.float16),
        })
    return in_maps


def run(x, Wq, Wk, Wv, Wo, trace=False):
    from concourse.bass_utils import run_bass_kernel_spmd

    nc = _get_program()
    in_maps = _make_in_maps(x, Wq, Wk, Wv, Wo)
    br = run_bass_kernel_spmd(nc, in_maps, list(range(N_CORES)), trace=trace)
    out = np.zeros((B, S, E), dtype=np.float32)
    for c in range(N_CORES):
        out[c // CPB] += br.results[c]["y"]
    return out, br


def kernel(x, Wq, Wk, Wv, Wo):
    out, _ = run(x, Wq, Wk, Wv, Wo, trace=False)
    return out
